# revision 36
# baseline (speedup 1.0000x reference)
"""Trainium2 Bass kernel for nn_HandGNNEncoder (2-layer GCN on 21-node hand
graphs + mean pool), data-parallel over 8 NeuronCores.

Math restructure (exact):
  reference: h1 = relu(A @ (x @ W1) + b1); out = mean_t(A @ (h1 @ W2) + b2)
  mean-pool is linear and commutes with W2: with m[s] = column-mean of A
  (all > 0) and m folded inside the relu (m*relu(z) = relu(m*z)):
      pooled[g,f] = sum_s relu(zm[g,s,f]),   zm = m[s]*(A(xW1)+b1)[s,f]
      out[g]      = pooled[g] @ W2 + b2   (b2 added on host)
  Stage 1 (PE): zm[(s,f), g] = TW.T @ x'[g], TW[(s',c),(s,f)] =
      m[s]*A[s,s']*W1[c,f]; b1 rides a constant-1 input row.  11 k-tiles
      of 128 (s,f)-columns.  K=43 <= 64, so stage 1 runs 2x ROW-TILED:
      tile T0 (SBUF partitions 0-42) computes even chunks while tile T8
      (partitions 64-106) concurrently computes odd chunks — measured 2x
      matmul throughput.  Each (k, chunk-pair) writes one [128,1024] PSUM
      pair tile (even chunk left half / odd chunk right half, 2 banks).
  Pooling: relu+accumulate on [128,1024] pair ops: DVE scalar_tensor_tensor
      chains (accA), ACT relu -> temps added by gpsimd/DVE (accB),
      merged to f16 pooled on DVE.
  Stage 2 (PE, full 128x128): one matmul per chunk, W2STACK[p,d] =
      W2[p%64,d]; psum pair-copied to f16 by ACT; DMA out per pair.

Input packing (host): xtp [128, G_CORE/2] holds even chunks' x'T on
partitions 0-42 and odd chunks' on partitions 64-106, chunk-pair p at
columns [512p:512(p+1)).  twp duplicates TW on both partition groups.
"""

import numpy as np

import concourse.bass as bass
import concourse.mybir as mybir
import concourse.tile as tile
from concourse import bass_utils

# ---- hardcoded problem constants ----
B, S, NNODE, CIN = 64, 512, 21, 2
D1, D2 = 64, 128
G = B * S                      # 32768 graphs
N_CORES = 8
G_CORE = G // N_CORES          # 4096 graphs per core
CHUNK = 512                    # graphs per chunk (one PSUM bank)
N_CHUNKS = G_CORE // CHUNK     # 8
N_PAIRS = N_CHUNKS // 2        # 4 chunk-pairs
K1 = NNODE * CIN + 1           # 43 contraction rows (42 feats + ones row)
KT = 11                        # k-tiles: 1408 / 128
M1 = KT * 128                  # 1344 (s,f) cols + 64 zero pad
N_PASSES = 2                   # pairs per pass = 2

EDGES = np.array(
    [[0, 1], [1, 2], [2, 3], [3, 4], [0, 5], [5, 6], [6, 7], [7, 8],
     [0, 9], [9, 10], [10, 11], [11, 12], [0, 13], [13, 14], [14, 15],
     [15, 16], [0, 17], [17, 18], [18, 19], [19, 20], [5, 9], [9, 13],
     [13, 17]], dtype=np.int64)


def fold_weights(W1, b1, W2):
    """Fold adjacency + mean-pool + b1 into TW; stack W2 for stage 2."""
    W1 = np.asarray(W1, np.float32)
    b1 = np.asarray(b1, np.float32)
    W2 = np.asarray(W2, np.float32)
    A = np.eye(NNODE, dtype=np.float32)
    A[EDGES[:, 1], EDGES[:, 0]] = 1.0
    deg = A.sum(axis=1)
    dis = 1.0 / np.sqrt(deg)
    a_norm = dis[:, None] * A * dis[None, :]          # [t, s]
    m = a_norm.mean(axis=0)                           # [21], all > 0

    tw = np.zeros((K1, M1), np.float32)
    blk = np.einsum("s,st,cf->tcsf", m, a_norm, W1)   # [s'=t, c, s, f]
    tw[: NNODE * CIN, : NNODE * D1] = blk.reshape(NNODE * CIN, NNODE * D1)
    tw[K1 - 1, : NNODE * D1] = (m[:, None] * b1[None, :]).reshape(-1)

    w2stack = np.concatenate([W2, W2], axis=0)        # [128, 128]
    return tw, w2stack


def build_bass(post=True, strip=True):
    f16 = mybir.dt.float16
    f32 = mybir.dt.float32
    nc = bass.Bass("TRN2", target_bir_lowering=False, debug=False)
    # xtp packs even chunks on partitions 0:43, odd chunks on 64:107
    xt_d = nc.dram_tensor("xt", [128, G_CORE // 2], f16,
                          kind="ExternalInput").ap()
    tw_d = nc.dram_tensor("tw", [128, M1], f16, kind="ExternalInput").ap()
    w2_d = nc.dram_tensor("w2", [D2, D2], f16, kind="ExternalInput").ap()
    out_d = nc.dram_tensor("out", [D2, G_CORE], f16, kind="ExternalOutput").ap()

    relu = mybir.ActivationFunctionType.Relu
    mx = mybir.AluOpType.max
    add = mybir.AluOpType.add
    PPP = N_PAIRS // N_PASSES             # pairs per pass

    # EW assignment (chunk-pair granularity, [128,1024] ops), phase-flipped
    # by pair parity so each k-step feeds ONE pair-op to DVE and one to ACT
    # (same-engine k-sets leave engines alternately idle/overloaded):
    #   pair phase 0: DVE drains k even (k0 init-max, rest stt into accA);
    #                 ACT drains k odd (k1 init-relu accB, k3/k5 -> gpsimd
    #                 ping-pong adds, k7/k9 -> stage-2 direct matmuls)
    #   pair phase 1: mirrored (DVE odd + k10, ACT even, k2/k4 gp adds,
    #                 k6/k8 stage-2 direct)
    #   merges (pooled = accA+accB) on gpsimd, copies on ACT
    def ew_plan(ph):
        return dict(dve_init=0, dve_stt=(2, 4, 6, 8),
                    act_init=1, gp_rt=(3,), dve_rt=(5, 7), s2_rt=(9, 10))

    with tile.TileContext(nc) as tc:
        with (
            tc.tile_pool(name="w", bufs=1) as wpool,
            tc.tile_pool(name="a", bufs=1) as apool,
            tc.tile_pool(name="r", bufs=6) as rpool,
            tc.tile_pool(name="m", bufs=2) as mpool,
            tc.tile_pool(name="o", bufs=2) as opool,
            tc.tile_pool(name="p1", bufs=3, space="PSUM") as p1pool,
            tc.tile_pool(name="p2", bufs=1, space="PSUM") as p2pool,
        ):
            tw_t = wpool.tile([128, M1], f16, tag="tw")
            xt_t = wpool.tile([128, G_CORE // 2], f16, tag="xt")
            w2_t = wpool.tile([D2, D2], f16, tag="w2")
            # one dma_start = one DMA engine (~22.5 GB/s); parallelize by
            # issuing many.  SP: per-pair xt slices (top+bottom halves);
            # ACT: tw + w2.
            for p in range(N_PAIRS):
                cs = p * CHUNK
                nc.sync.dma_start(out=xt_t[0:K1, cs:cs + CHUNK],
                                  in_=xt_d[0:K1, cs:cs + CHUNK])
                nc.sync.dma_start(out=xt_t[64:64 + K1, cs:cs + CHUNK],
                                  in_=xt_d[64:64 + K1, cs:cs + CHUNK])
            nc.scalar.dma_start(out=tw_t[0:K1, :512], in_=tw_d[0:K1, :512])
            nc.scalar.dma_start(out=tw_t[64:64 + K1, :512],
                                in_=tw_d[64:64 + K1, :512])
            nc.scalar.dma_start(out=tw_t[0:K1, 512:], in_=tw_d[0:K1, 512:])
            nc.scalar.dma_start(out=tw_t[64:64 + K1, 512:],
                                in_=tw_d[64:64 + K1, 512:])
            nc.scalar.dma_start(out=w2_t, in_=w2_d)

            for ps in range(N_PASSES):
                prs = range(ps * PPP, (ps + 1) * PPP)
                accA = {p: apool.tile([128, 2 * CHUNK], f16,
                                      name=f"accA{p}", tag=f"aA{p % PPP}")
                        for p in prs}
                accB = {p: [apool.tile([128, 2 * CHUNK], f16,
                                       name=f"accB{p}_{i}",
                                       tag=f"aB{p % PPP}_{i}")
                            for i in range(2)]
                        for p in prs}
                nB = {p: 0 for p in prs}
                s2rt = {p: [] for p in prs}

                for k in range(KT):
                    kc = slice(k * 128, (k + 1) * 128)
                    # group matmuls by row-tile so same-group Ldweights are
                    # consecutive (strippable) and cross-group ones pull
                    # ahead; disjoint-group matmuls still overlap in HW
                    pts = {}
                    for p in prs:
                        cs = p * CHUNK
                        pts[p] = p1pool.tile([128, 2 * CHUNK], f32, tag="p1",
                                             name=f"pt{k}_{p}")
                        nc.tensor.matmul(
                            pts[p][:, :CHUNK], lhsT=tw_t[0:K1, kc],
                            rhs=xt_t[0:K1, cs:cs + CHUNK],
                            start=True, stop=True, tile_position=(0, 0))
                    for p in prs:
                        cs = p * CHUNK
                        nc.tensor.matmul(
                            pts[p][:, CHUNK:], lhsT=tw_t[64:64 + K1, kc],
                            rhs=xt_t[64:64 + K1, cs:cs + CHUNK],
                            start=True, stop=True, tile_position=(64, 0))
                    for p in prs:
                        pt = pts[p]
                        plan = ew_plan(p % 2)
                        if k == plan["dve_init"]:
                            nc.vector.tensor_scalar_max(
                                out=accA[p], in0=pt, scalar1=0.0)
                        elif k == plan["act_init"]:
                            nc.scalar.activation(out=accB[p][0], in_=pt,
                                                 func=relu)
                        elif k in plan["dve_stt"]:
                            nc.vector.scalar_tensor_tensor(
                                out=accA[p], in0=pt, scalar=0.0,
                                in1=accA[p], op0=mx, op1=add)
                        elif k in plan["s2_rt"]:
                            rt = rpool.tile([128, 2 * CHUNK], f16,
                                            tag=f"s2rt{k % 2}",
                                            name=f"s2rt{k}_{p}")
                            nc.scalar.activation(out=rt, in_=pt, func=relu)
                            s2rt[p].append(rt)
                        else:
                            gp = k in plan["gp_rt"]
                            # separate pools per consumer: a slow gpsimd add
                            # must not block ACT's other relu temps (WAR on
                            # a shared pool stalls the whole drain pipeline)
                            rt = rpool.tile([128, 2 * CHUNK], f16,
                                            tag="rtg" if gp else "rtv")
                            nc.scalar.activation(out=rt, in_=pt, func=relu)
                            i = nB[p]
                            eng = nc.gpsimd if gp else nc.vector
                            # gpsimd crashes on in-place; ping-pong
                            eng.tensor_tensor(
                                out=accB[p][(i + 1) % 2],
                                in0=accB[p][i % 2], in1=rt, op=add)
                            nB[p] = i + 1
                for p in prs:
                    pooled = mpool.tile([128, 2 * CHUNK], f16,
                                        name=f"pool{p}", tag="pool")
                    nc.vector.tensor_tensor(
                        out=pooled, in0=accA[p], in1=accB[p][nB[p] % 2],
                        op=add)
                    ops = p2pool.tile([D2, 2 * CHUNK], f32, tag="p2")
                    rhss = [pooled] + s2rt[p]
                    for i, rh in enumerate(rhss):
                        st = i == 0
                        sp_ = i == len(rhss) - 1
                        nc.tensor.matmul(ops[:, :CHUNK], lhsT=w2_t,
                                         rhs=rh[:, :CHUNK], start=st,
                                         stop=sp_, skip_group_check=True)
                        nc.tensor.matmul(ops[:, CHUNK:], lhsT=w2_t,
                                         rhs=rh[:, CHUNK:], start=st,
                                         stop=sp_, skip_group_check=True)
                    ot = opool.tile([D2, 2 * CHUNK], f16, name=f"ot{p}",
                                    tag="ot")
                    nc.scalar.copy(out=ot, in_=ops)
                    cs = p * 2 * CHUNK
                    # split by rows across SP+ACT queues; finer on the last
                    nsplit = 4 if p == N_PAIRS - 1 else 2
                    rstep = D2 // nsplit
                    for j, r in enumerate(range(0, D2, rstep)):
                        nc.sync.dma_start(
                            out=out_d[r:r + rstep, cs:cs + 2 * CHUNK],
                            in_=ot[r:r + rstep])

    if post:
        if strip:
            _strip_redundant_ldweights(nc)
        _rebalance_matmul_waits(nc)
    return nc


def _ap_key(ap):
    return (ap.memref, ap.offset, tuple(tuple(d) for d in ap.ap))


def _strip_redundant_ldweights(nc):
    """Consecutive matmuls on the same stationary tile don't need to reload
    the PE array. Weights are tracked per row-group (row tiling keeps an
    independent stationary set per group). Any sync waits a dropped load
    held move onto the next kept PE instruction."""
    import bass_rust

    for fn in nc.m.functions:
        for blk in fn.blocks:
            insts = list(blk.instructions)
            out = []
            last_key = None
            carry_waits = []
            for inst in insts:
                tn = type(inst).__name__
                if tn == "InstLdweights":
                    tp = getattr(inst, "tile_position", None)
                    # only drop an exact repeat of the IMMEDIATELY previous
                    # load: an intervening other-row-group load invalidates
                    # this group's stationary on real HW
                    key = (tuple(tp) if tp else None, _ap_key(inst.ins[0]))
                    if key == last_key:
                        si = inst.sync_info
                        if si is not None:
                            carry_waits.extend(si.on_wait)
                            assert not si.on_update, (
                                "won't drop ldweights holding sem updates")
                        continue  # drop the instruction
                    last_key = key
                elif tn == "InstMatmult" and carry_waits:
                    si = inst.sync_info
                    waits = list(si.on_wait) if si else []
                    ups = list(si.on_update) if si else []
                    inst.sync_info = bass_rust.SyncInfo(
                        on_wait=carry_waits + waits, on_update=ups)
                    carry_waits = []
                out.append(inst)
            assert not carry_waits
            if len(out) != len(insts):
                blk.instructions = out


def _rebalance_matmul_waits(nc):
    """Walrus' TPB ISA structs accept only one sync-wait per instruction on
    the compute engines, but Tile can attach several. Keep one wait on the
    instruction and move the excess onto freshly inserted same-engine NoOps
    just before it (same in-order queue => same or stronger ordering).
    Never park waits on Ldweights: a wait there blocks the PE's
    pull-ahead that otherwise hides the weight load entirely."""
    import bass_rust

    exempt = {"InstEventSemaphore", "InstUnconditionalBranch",
              "InstCall", "InstISA", "InstNoOp"}
    nop_ctr = [0]
    for fn in nc.m.functions:
        for blk in fn.blocks:
            insts = list(blk.instructions)
            out = []
            for inst in insts:
                tn = type(inst).__name__
                si = inst.sync_info
                nw = len(si.on_wait) if si is not None else 0
                if tn in exempt or nw <= 1:
                    out.append(inst)
                    continue
                waits = list(si.on_wait)
                moved, kept = waits[:-1], waits[-1:]
                for w in moved:
                    nop_ctr[0] += 1
                    nop = mybir.InstNoOp(
                        name=f"I-waitnop-{nop_ctr[0]}", ins=[], outs=[])
                    nop.engine = inst.engine
                    nop.sync_info = bass_rust.SyncInfo(
                        on_wait=[w], on_update=[])
                    out.append(nop)
                inst.sync_info = bass_rust.SyncInfo(
                    on_wait=kept, on_update=list(si.on_update))
                out.append(inst)
            if len(out) != len(insts):
                blk.instructions = out


_NC_CACHE = None


def _get_nc():
    global _NC_CACHE
    if _NC_CACHE is None:
        import os
        strip = os.environ.get("KERNEL_NO_STRIP", "0") != "1"
        _NC_CACHE = build_bass(strip=strip)
    return _NC_CACHE


def make_in_maps(hand_landmarks, W1, b1, W2, b2, np_dt=np.float16):
    tw, w2stack = fold_weights(W1, b1, W2)
    twp = np.zeros((128, M1), np_dt)
    twp[0:K1] = tw
    twp[64:64 + K1] = tw
    w2stack = w2stack.astype(np_dt)
    x = np.asarray(hand_landmarks, np.float32).reshape(G, NNODE * CIN)
    xt = np.empty((K1, G), np.float32)
    xt[: NNODE * CIN] = x.T
    xt[K1 - 1] = 1.0
    in_maps = []
    for i in range(N_CORES):
        xc = xt[:, i * G_CORE:(i + 1) * G_CORE]
        xtp = np.zeros((128, G_CORE // 2), np_dt)
        # even chunks -> partitions 0:43, odd chunks -> 64:107
        for p in range(N_PAIRS):
            xtp[0:K1, p * CHUNK:(p + 1) * CHUNK] = \
                xc[:, (2 * p) * CHUNK:(2 * p + 1) * CHUNK]
            xtp[64:64 + K1, p * CHUNK:(p + 1) * CHUNK] = \
                xc[:, (2 * p + 1) * CHUNK:(2 * p + 2) * CHUNK]
        in_maps.append({"xt": xtp, "tw": twp, "w2": w2stack})
    return in_maps


def gather_out(results, b2):
    full = np.concatenate([results[i]["out"] for i in range(N_CORES)], axis=1)
    out = full.T.astype(np.float32) + np.asarray(b2, np.float32)[None, :]
    return np.ascontiguousarray(out).reshape(B, S, D2)


def run(in_maps, trace=False, **kw):
    res = bass_utils.run_bass_kernel_spmd(
        _get_nc(), in_maps, core_ids=list(range(N_CORES)), trace=trace, **kw)
    return res


def kernel(hand_landmarks, W1, b1, W2, b2):
    in_maps = make_in_maps(hand_landmarks, W1, b1, W2, b2)
    res = run(in_maps)
    return gather_out(res.results, b2)


# revision 38
# speedup vs baseline: 1.1130x; 1.1130x over previous
"""Trainium2 Bass kernel for nn_HandGNNEncoder (2-layer GCN on 21-node hand
graphs + mean pool), data-parallel over 8 NeuronCores.

Math restructure (exact):
  reference: h1 = relu(A @ (x @ W1) + b1); out = mean_t(A @ (h1 @ W2) + b2)
  mean-pool is linear and commutes with W2: with m[s] = column-mean of A
  (all > 0) and m folded inside the relu (m*relu(z) = relu(m*z)):
      pooled[g,f] = sum_s relu(zm[g,s,f]),   zm = m[s]*(A(xW1)+b1)[s,f]
      out[g]      = pooled[g] @ W2 + b2   (b2 added on host)
  Stage 1 (PE): zm[(s,f), g] = TW.T @ x'[g], TW[(s',c),(s,f)] =
      m[s]*A[s,s']*W1[c,f]; b1 rides a constant-1 input row.  11 k-tiles
      of 128 (s,f)-columns.  K=43 <= 64, so stage 1 runs 2x ROW-TILED:
      tile T0 (SBUF partitions 0-42) computes even chunks while tile T8
      (partitions 64-106) concurrently computes odd chunks — measured 2x
      matmul throughput.  Each (k, chunk-pair) writes one [128,1024] PSUM
      pair tile (even chunk left half / odd chunk right half, 2 banks).
  Pooling: relu+accumulate on [128,1024] pair ops: DVE scalar_tensor_tensor
      chains (accA), ACT relu -> temps added by gpsimd/DVE (accB),
      merged to f16 pooled on DVE.
  Stage 2 (PE, full 128x128): one matmul per chunk, W2STACK[p,d] =
      W2[p%64,d]; psum pair-copied to f16 by ACT; DMA out per pair.

Input packing (host): xtp [128, G_CORE/2] holds even chunks' x'T on
partitions 0-42 and odd chunks' on partitions 64-106, chunk-pair p at
columns [512p:512(p+1)).  twp duplicates TW on both partition groups.
"""

import numpy as np

import concourse.bass as bass
import concourse.mybir as mybir
import concourse.tile as tile
from concourse import bass_utils

# ---- hardcoded problem constants ----
B, S, NNODE, CIN = 64, 512, 21, 2
D1, D2 = 64, 128
G = B * S                      # 32768 graphs
N_CORES = 8
G_CORE = G // N_CORES          # 4096 graphs per core
CHUNK = 512                    # graphs per chunk (one PSUM bank)
N_CHUNKS = G_CORE // CHUNK     # 8
N_PAIRS = N_CHUNKS // 2        # 4 chunk-pairs
K1 = NNODE * CIN + 1           # 43 contraction rows (42 feats + ones row)
KT = 11                        # k-tiles: 1408 / 128
M1 = KT * 128                  # 1344 (s,f) cols + 64 zero pad
N_PASSES = 2                   # pairs per pass = 2

EDGES = np.array(
    [[0, 1], [1, 2], [2, 3], [3, 4], [0, 5], [5, 6], [6, 7], [7, 8],
     [0, 9], [9, 10], [10, 11], [11, 12], [0, 13], [13, 14], [14, 15],
     [15, 16], [0, 17], [17, 18], [18, 19], [19, 20], [5, 9], [9, 13],
     [13, 17]], dtype=np.int64)


def fold_weights(W1, b1, W2):
    """Fold adjacency + mean-pool + b1 into TW; stack W2 for stage 2."""
    W1 = np.asarray(W1, np.float32)
    b1 = np.asarray(b1, np.float32)
    W2 = np.asarray(W2, np.float32)
    A = np.eye(NNODE, dtype=np.float32)
    A[EDGES[:, 1], EDGES[:, 0]] = 1.0
    deg = A.sum(axis=1)
    dis = 1.0 / np.sqrt(deg)
    a_norm = dis[:, None] * A * dis[None, :]          # [t, s]
    m = a_norm.mean(axis=0)                           # [21], all > 0

    tw = np.zeros((K1, M1), np.float32)
    blk = np.einsum("s,st,cf->tcsf", m, a_norm, W1)   # [s'=t, c, s, f]
    tw[: NNODE * CIN, : NNODE * D1] = blk.reshape(NNODE * CIN, NNODE * D1)
    tw[K1 - 1, : NNODE * D1] = (m[:, None] * b1[None, :]).reshape(-1)

    w2stack = np.concatenate([W2, W2], axis=0)        # [128, 128]
    return tw, w2stack


def build_bass(post=True, strip=True):
    f16 = mybir.dt.float16
    f32 = mybir.dt.float32
    nc = bass.Bass("TRN2", target_bir_lowering=False, debug=False)
    # xtp packs even chunks on partitions 0:43, odd chunks on 64:107
    xt_d = nc.dram_tensor("xt", [128, G_CORE // 2], f16,
                          kind="ExternalInput").ap()
    tw_d = nc.dram_tensor("tw", [128, M1], f16, kind="ExternalInput").ap()
    w2_d = nc.dram_tensor("w2", [D2, D2], f16, kind="ExternalInput").ap()
    out_d = nc.dram_tensor("out", [D2, G_CORE], f16, kind="ExternalOutput").ap()

    relu = mybir.ActivationFunctionType.Relu
    mx = mybir.AluOpType.max
    add = mybir.AluOpType.add
    PPP = N_PAIRS // N_PASSES             # pairs per pass

    # EW assignment (chunk-pair granularity, [128,1024] ops), phase-flipped
    # by pair parity so each k-step feeds ONE pair-op to DVE and one to ACT
    # (same-engine k-sets leave engines alternately idle/overloaded):
    #   pair phase 0: DVE drains k even (k0 init-max, rest stt into accA);
    #                 ACT drains k odd (k1 init-relu accB, k3/k5 -> gpsimd
    #                 ping-pong adds, k7/k9 -> stage-2 direct matmuls)
    #   pair phase 1: mirrored (DVE odd + k10, ACT even, k2/k4 gp adds,
    #                 k6/k8 stage-2 direct)
    #   merges (pooled = accA+accB) on gpsimd, copies on ACT
    def ew_plan(ph):
        return dict(dve_init=0, dve_stt=(2, 4, 6, 8),
                    act_init=1, gp_rt=(3,), dve_rt=(5, 7), s2_rt=(9, 10))

    with tile.TileContext(nc) as tc:
        with (
            tc.tile_pool(name="w", bufs=1) as wpool,
            tc.tile_pool(name="a", bufs=1) as apool,
            tc.tile_pool(name="r", bufs=6) as rpool,
            tc.tile_pool(name="m", bufs=2) as mpool,
            tc.tile_pool(name="o", bufs=2) as opool,
            tc.tile_pool(name="p1", bufs=3, space="PSUM") as p1pool,
            tc.tile_pool(name="p2", bufs=1, space="PSUM") as p2pool,
        ):
            tw_t = wpool.tile([128, M1], f16, tag="tw")
            xt_t = wpool.tile([128, G_CORE // 2], f16, tag="xt")
            w2_t = wpool.tile([D2, D2], f16, tag="w2")
            # one dma_start = one DMA engine (~22.5 GB/s); parallelize by
            # issuing many.  SP: per-pair xt slices (top+bottom halves);
            # ACT: tw + w2.
            for p in range(N_PAIRS):
                cs = p * CHUNK
                nc.sync.dma_start(out=xt_t[0:K1, cs:cs + CHUNK],
                                  in_=xt_d[0:K1, cs:cs + CHUNK])
                nc.sync.dma_start(out=xt_t[64:64 + K1, cs:cs + CHUNK],
                                  in_=xt_d[64:64 + K1, cs:cs + CHUNK])
            nc.scalar.dma_start(out=tw_t[0:K1, :128], in_=tw_d[0:K1, :128])
            nc.scalar.dma_start(out=tw_t[64:64 + K1, :128],
                                in_=tw_d[64:64 + K1, :128])
            nc.scalar.dma_start(out=tw_t[0:K1, 128:768],
                                in_=tw_d[0:K1, 128:768])
            nc.scalar.dma_start(out=tw_t[64:64 + K1, 128:768],
                                in_=tw_d[64:64 + K1, 128:768])
            nc.scalar.dma_start(out=tw_t[0:K1, 768:], in_=tw_d[0:K1, 768:])
            nc.scalar.dma_start(out=tw_t[64:64 + K1, 768:],
                                in_=tw_d[64:64 + K1, 768:])
            nc.scalar.dma_start(out=w2_t, in_=w2_d)

            PASS_PAIRS = [(0, 1, 2), (3,)]
            for ps in range(N_PASSES):
                prs = PASS_PAIRS[ps]
                accA = {p: apool.tile([128, 2 * CHUNK], f16,
                                      name=f"accA{p}", tag=f"aA{p % 3}")
                        for p in prs}
                accB = {p: [apool.tile([128, 2 * CHUNK], f16,
                                       name=f"accB{p}_{i}",
                                       tag=f"aB{p % 3}_{i}")
                            for i in range(2)]
                        for p in prs}
                nB = {p: 0 for p in prs}
                s2rt = {p: [] for p in prs}

                for k in range(KT):
                    kc = slice(k * 128, (k + 1) * 128)
                    for p in prs:
                        cs = p * CHUNK
                        pt = p1pool.tile([128, 2 * CHUNK], f32, tag="p1",
                                         name=f"pt{k}_{p}")
                        nc.tensor.matmul(
                            pt[:, :CHUNK], lhsT=tw_t[0:K1, kc],
                            rhs=xt_t[0:K1, cs:cs + CHUNK],
                            start=True, stop=True, tile_position=(0, 0))
                        nc.tensor.matmul(
                            pt[:, CHUNK:], lhsT=tw_t[64:64 + K1, kc],
                            rhs=xt_t[64:64 + K1, cs:cs + CHUNK],
                            start=True, stop=True, tile_position=(64, 0))
                        plan = ew_plan(p % 2)
                        if k == plan["dve_init"]:
                            nc.vector.tensor_scalar_max(
                                out=accA[p], in0=pt, scalar1=0.0)
                        elif k == plan["act_init"]:
                            nc.scalar.activation(out=accB[p][0], in_=pt,
                                                 func=relu)
                        elif k in plan["dve_stt"]:
                            nc.vector.scalar_tensor_tensor(
                                out=accA[p], in0=pt, scalar=0.0,
                                in1=accA[p], op0=mx, op1=add)
                        elif k in plan["s2_rt"]:
                            rt = rpool.tile([128, 2 * CHUNK], f16,
                                            tag=f"s2rt{k % 2}",
                                            name=f"s2rt{k}_{p}")
                            nc.scalar.activation(out=rt, in_=pt, func=relu)
                            s2rt[p].append(rt)
                        else:
                            gp = k in plan["gp_rt"]
                            # separate pools per consumer: a slow gpsimd add
                            # must not block ACT's other relu temps (WAR on
                            # a shared pool stalls the whole drain pipeline)
                            rt = rpool.tile([128, 2 * CHUNK], f16,
                                            tag="rtg" if gp else "rtv")
                            nc.scalar.activation(out=rt, in_=pt, func=relu)
                            i = nB[p]
                            eng = nc.gpsimd if gp else nc.vector
                            # gpsimd crashes on in-place; ping-pong
                            eng.tensor_tensor(
                                out=accB[p][(i + 1) % 2],
                                in0=accB[p][i % 2], in1=rt, op=add)
                            nB[p] = i + 1
                for p in prs:
                    pooled = mpool.tile([128, 2 * CHUNK], f16,
                                        name=f"pool{p}", tag="pool")
                    nc.vector.tensor_tensor(
                        out=pooled, in0=accA[p], in1=accB[p][nB[p] % 2],
                        op=add)
                    ops = p2pool.tile([D2, 2 * CHUNK], f32, tag="p2")
                    rhss = [pooled] + s2rt[p]
                    for i, rh in enumerate(rhss):
                        st = i == 0
                        sp_ = i == len(rhss) - 1
                        nc.tensor.matmul(ops[:, :CHUNK], lhsT=w2_t,
                                         rhs=rh[:, :CHUNK], start=st,
                                         stop=sp_, skip_group_check=True)
                        nc.tensor.matmul(ops[:, CHUNK:], lhsT=w2_t,
                                         rhs=rh[:, CHUNK:], start=st,
                                         stop=sp_, skip_group_check=True)
                    ot = opool.tile([D2, 2 * CHUNK], f16, name=f"ot{p}",
                                    tag="ot")
                    nc.scalar.copy(out=ot[:, :CHUNK], in_=ops[:, :CHUNK])
                    nc.vector.tensor_copy(out=ot[:, CHUNK:],
                                          in_=ops[:, CHUNK:])
                    cs = p * 2 * CHUNK
                    # split by rows across SP+ACT queues; finer on the last
                    nsplit = 4 if p == N_PAIRS - 1 else 2
                    rstep = D2 // nsplit
                    for j, r in enumerate(range(0, D2, rstep)):
                        nc.sync.dma_start(
                            out=out_d[r:r + rstep, cs:cs + 2 * CHUNK],
                            in_=ot[r:r + rstep])

    if post:
        if strip:
            _strip_redundant_ldweights(nc)
        _rebalance_matmul_waits(nc)
    return nc


def _ap_key(ap):
    return (ap.memref, ap.offset, tuple(tuple(d) for d in ap.ap))


def _strip_redundant_ldweights(nc):
    """Consecutive matmuls on the same stationary tile don't need to reload
    the PE array. Weights are tracked per row-group (row tiling keeps an
    independent stationary set per group). Any sync waits a dropped load
    held move onto the next kept PE instruction."""
    import bass_rust

    for fn in nc.m.functions:
        for blk in fn.blocks:
            insts = list(blk.instructions)
            out = []
            last_key = None
            carry_waits = []
            for inst in insts:
                tn = type(inst).__name__
                if tn == "InstLdweights":
                    tp = getattr(inst, "tile_position", None)
                    # only drop an exact repeat of the IMMEDIATELY previous
                    # load: an intervening other-row-group load invalidates
                    # this group's stationary on real HW
                    key = (tuple(tp) if tp else None, _ap_key(inst.ins[0]))
                    if key == last_key:
                        si = inst.sync_info
                        if si is not None:
                            carry_waits.extend(si.on_wait)
                            assert not si.on_update, (
                                "won't drop ldweights holding sem updates")
                        continue  # drop the instruction
                    last_key = key
                elif tn == "InstMatmult" and carry_waits:
                    si = inst.sync_info
                    waits = list(si.on_wait) if si else []
                    ups = list(si.on_update) if si else []
                    inst.sync_info = bass_rust.SyncInfo(
                        on_wait=carry_waits + waits, on_update=ups)
                    carry_waits = []
                out.append(inst)
            assert not carry_waits
            if len(out) != len(insts):
                blk.instructions = out


def _rebalance_matmul_waits(nc):
    """Walrus' TPB ISA structs accept only one sync-wait per instruction on
    the compute engines, but Tile can attach several. Keep one wait on the
    instruction and move the excess onto freshly inserted same-engine NoOps
    just before it (same in-order queue => same or stronger ordering).
    Never park waits on Ldweights: a wait there blocks the PE's
    pull-ahead that otherwise hides the weight load entirely."""
    import bass_rust

    exempt = {"InstEventSemaphore", "InstUnconditionalBranch",
              "InstCall", "InstISA", "InstNoOp"}
    nop_ctr = [0]
    for fn in nc.m.functions:
        for blk in fn.blocks:
            insts = list(blk.instructions)
            out = []
            for inst in insts:
                tn = type(inst).__name__
                si = inst.sync_info
                nw = len(si.on_wait) if si is not None else 0
                if tn in exempt or nw <= 1:
                    out.append(inst)
                    continue
                waits = list(si.on_wait)
                moved, kept = waits[:-1], waits[-1:]
                for w in moved:
                    nop_ctr[0] += 1
                    nop = mybir.InstNoOp(
                        name=f"I-waitnop-{nop_ctr[0]}", ins=[], outs=[])
                    nop.engine = inst.engine
                    nop.sync_info = bass_rust.SyncInfo(
                        on_wait=[w], on_update=[])
                    out.append(nop)
                inst.sync_info = bass_rust.SyncInfo(
                    on_wait=kept, on_update=list(si.on_update))
                out.append(inst)
            if len(out) != len(insts):
                blk.instructions = out


_NC_CACHE = None


def _get_nc():
    global _NC_CACHE
    if _NC_CACHE is None:
        import os
        strip = os.environ.get("KERNEL_NO_STRIP", "0") != "1"
        _NC_CACHE = build_bass(strip=strip)
    return _NC_CACHE


def make_in_maps(hand_landmarks, W1, b1, W2, b2, np_dt=np.float16):
    tw, w2stack = fold_weights(W1, b1, W2)
    twp = np.zeros((128, M1), np_dt)
    twp[0:K1] = tw
    twp[64:64 + K1] = tw
    w2stack = w2stack.astype(np_dt)
    x = np.asarray(hand_landmarks, np.float32).reshape(G, NNODE * CIN)
    xt = np.empty((K1, G), np.float32)
    xt[: NNODE * CIN] = x.T
    xt[K1 - 1] = 1.0
    in_maps = []
    for i in range(N_CORES):
        xc = xt[:, i * G_CORE:(i + 1) * G_CORE]
        xtp = np.zeros((128, G_CORE // 2), np_dt)
        # even chunks -> partitions 0:43, odd chunks -> 64:107
        for p in range(N_PAIRS):
            xtp[0:K1, p * CHUNK:(p + 1) * CHUNK] = \
                xc[:, (2 * p) * CHUNK:(2 * p + 1) * CHUNK]
            xtp[64:64 + K1, p * CHUNK:(p + 1) * CHUNK] = \
                xc[:, (2 * p + 1) * CHUNK:(2 * p + 2) * CHUNK]
        in_maps.append({"xt": xtp, "tw": twp, "w2": w2stack})
    return in_maps


def gather_out(results, b2):
    full = np.concatenate([results[i]["out"] for i in range(N_CORES)], axis=1)
    out = full.T.astype(np.float32) + np.asarray(b2, np.float32)[None, :]
    return np.ascontiguousarray(out).reshape(B, S, D2)


def run(in_maps, trace=False, **kw):
    res = bass_utils.run_bass_kernel_spmd(
        _get_nc(), in_maps, core_ids=list(range(N_CORES)), trace=trace, **kw)
    return res


def kernel(hand_landmarks, W1, b1, W2, b2):
    in_maps = make_in_maps(hand_landmarks, W1, b1, W2, b2)
    res = run(in_maps)
    return gather_out(res.results, b2)


# revision 39
# speedup vs baseline: 1.1524x; 1.0353x over previous
"""Trainium2 Bass kernel for nn_HandGNNEncoder (2-layer GCN on 21-node hand
graphs + mean pool), data-parallel over 8 NeuronCores.

Math restructure (exact):
  reference: h1 = relu(A @ (x @ W1) + b1); out = mean_t(A @ (h1 @ W2) + b2)
  mean-pool is linear and commutes with W2: with m[s] = column-mean of A
  (all > 0) and m folded inside the relu (m*relu(z) = relu(m*z)):
      pooled[g,f] = sum_s relu(zm[g,s,f]),   zm = m[s]*(A(xW1)+b1)[s,f]
      out[g]      = pooled[g] @ W2 + b2   (b2 added on host)
  Stage 1 (PE): zm[(s,f), g] = TW.T @ x'[g], TW[(s',c),(s,f)] =
      m[s]*A[s,s']*W1[c,f]; b1 rides a constant-1 input row.  11 k-tiles
      of 128 (s,f)-columns.  K=43 <= 64, so stage 1 runs 2x ROW-TILED:
      tile T0 (SBUF partitions 0-42) computes even chunks while tile T8
      (partitions 64-106) concurrently computes odd chunks — measured 2x
      matmul throughput.  Each (k, chunk-pair) writes one [128,1024] PSUM
      pair tile (even chunk left half / odd chunk right half, 2 banks).
  Pooling: relu+accumulate on [128,1024] pair ops: DVE scalar_tensor_tensor
      chains (accA), ACT relu -> temps added by gpsimd/DVE (accB),
      merged to f16 pooled on DVE.
  Stage 2 (PE, full 128x128): one matmul per chunk, W2STACK[p,d] =
      W2[p%64,d]; psum pair-copied to f16 by ACT; DMA out per pair.

Input packing (host): xtp [128, G_CORE/2] holds even chunks' x'T on
partitions 0-42 and odd chunks' on partitions 64-106, chunk-pair p at
columns [512p:512(p+1)).  twp duplicates TW on both partition groups.
"""

import numpy as np

import concourse.bass as bass
import concourse.mybir as mybir
import concourse.tile as tile
from concourse import bass_utils

# ---- hardcoded problem constants ----
B, S, NNODE, CIN = 64, 512, 21, 2
D1, D2 = 64, 128
G = B * S                      # 32768 graphs
N_CORES = 8
G_CORE = G // N_CORES          # 4096 graphs per core
CHUNK = 512                    # graphs per chunk (one PSUM bank)
N_CHUNKS = G_CORE // CHUNK     # 8
N_PAIRS = N_CHUNKS // 2        # 4 chunk-pairs
K1 = NNODE * CIN + 1           # 43 contraction rows (42 feats + ones row)
KT = 11                        # k-tiles: 1408 / 128
M1 = KT * 128                  # 1344 (s,f) cols + 64 zero pad
N_PASSES = 2                   # pairs per pass = 2

EDGES = np.array(
    [[0, 1], [1, 2], [2, 3], [3, 4], [0, 5], [5, 6], [6, 7], [7, 8],
     [0, 9], [9, 10], [10, 11], [11, 12], [0, 13], [13, 14], [14, 15],
     [15, 16], [0, 17], [17, 18], [18, 19], [19, 20], [5, 9], [9, 13],
     [13, 17]], dtype=np.int64)


def fold_weights(W1, b1, W2):
    """Fold adjacency + mean-pool + b1 into TW; stack W2 for stage 2."""
    W1 = np.asarray(W1, np.float32)
    b1 = np.asarray(b1, np.float32)
    W2 = np.asarray(W2, np.float32)
    A = np.eye(NNODE, dtype=np.float32)
    A[EDGES[:, 1], EDGES[:, 0]] = 1.0
    deg = A.sum(axis=1)
    dis = 1.0 / np.sqrt(deg)
    a_norm = dis[:, None] * A * dis[None, :]          # [t, s]
    m = a_norm.mean(axis=0)                           # [21], all > 0

    tw = np.zeros((K1, M1), np.float32)
    blk = np.einsum("s,st,cf->tcsf", m, a_norm, W1)   # [s'=t, c, s, f]
    tw[: NNODE * CIN, : NNODE * D1] = blk.reshape(NNODE * CIN, NNODE * D1)
    tw[K1 - 1, : NNODE * D1] = (m[:, None] * b1[None, :]).reshape(-1)

    w2stack = np.concatenate([W2, W2], axis=0)        # [128, 128]
    return tw, w2stack


def build_bass(post=True, strip=True):
    f16 = mybir.dt.float16
    f32 = mybir.dt.float32
    nc = bass.Bass("TRN2", target_bir_lowering=False, debug=False)
    # xtp packs even chunks on partitions 0:43, odd chunks on 64:107
    xt_d = nc.dram_tensor("xt", [128, G_CORE // 2], f16,
                          kind="ExternalInput").ap()
    tw_d = nc.dram_tensor("tw", [128, M1], f16, kind="ExternalInput").ap()
    w2_d = nc.dram_tensor("w2", [D2, D2], f16, kind="ExternalInput").ap()
    out_d = nc.dram_tensor("out", [D2, G_CORE], f16, kind="ExternalOutput").ap()

    relu = mybir.ActivationFunctionType.Relu
    mx = mybir.AluOpType.max
    add = mybir.AluOpType.add
    PPP = N_PAIRS // N_PASSES             # pairs per pass

    # EW assignment (chunk-pair granularity, [128,1024] ops), phase-flipped
    # by pair parity so each k-step feeds ONE pair-op to DVE and one to ACT
    # (same-engine k-sets leave engines alternately idle/overloaded):
    #   pair phase 0: DVE drains k even (k0 init-max, rest stt into accA);
    #                 ACT drains k odd (k1 init-relu accB, k3/k5 -> gpsimd
    #                 ping-pong adds, k7/k9 -> stage-2 direct matmuls)
    #   pair phase 1: mirrored (DVE odd + k10, ACT even, k2/k4 gp adds,
    #                 k6/k8 stage-2 direct)
    #   merges (pooled = accA+accB) on gpsimd, copies on ACT
    def ew_plan(ph):
        return dict(dve_init=0, dve_stt=(2, 4, 6, 8),
                    act_init=1, gp_rt=(3,), dve_rt=(5, 7), s2_rt=(9, 10))

    with tile.TileContext(nc) as tc:
        with (
            tc.tile_pool(name="w", bufs=1) as wpool,
            tc.tile_pool(name="a", bufs=1) as apool,
            tc.tile_pool(name="r", bufs=6) as rpool,
            tc.tile_pool(name="m", bufs=2) as mpool,
            tc.tile_pool(name="o", bufs=2) as opool,
            tc.tile_pool(name="p1", bufs=3, space="PSUM") as p1pool,
            tc.tile_pool(name="p2", bufs=1, space="PSUM") as p2pool,
        ):
            tw_t = wpool.tile([128, M1], f16, tag="tw")
            xt_t = wpool.tile([128, G_CORE // 2], f16, tag="xt")
            w2_t = wpool.tile([D2, D2], f16, tag="w2")
            # one dma_start = one DMA engine (~22.5 GB/s); parallelize by
            # issuing many.  SP: per-pair xt slices (top+bottom halves);
            # ACT: tw + w2.
            for p in range(N_PAIRS):
                cs = p * CHUNK
                nc.sync.dma_start(out=xt_t[0:K1, cs:cs + CHUNK],
                                  in_=xt_d[0:K1, cs:cs + CHUNK])
                nc.sync.dma_start(out=xt_t[64:64 + K1, cs:cs + CHUNK],
                                  in_=xt_d[64:64 + K1, cs:cs + CHUNK])
            nc.scalar.dma_start(out=tw_t[0:K1, :128], in_=tw_d[0:K1, :128])
            nc.scalar.dma_start(out=tw_t[64:64 + K1, :128],
                                in_=tw_d[64:64 + K1, :128])
            nc.scalar.dma_start(out=tw_t[0:K1, 128:768],
                                in_=tw_d[0:K1, 128:768])
            nc.scalar.dma_start(out=tw_t[64:64 + K1, 128:768],
                                in_=tw_d[64:64 + K1, 128:768])
            nc.scalar.dma_start(out=tw_t[0:K1, 768:], in_=tw_d[0:K1, 768:])
            nc.scalar.dma_start(out=tw_t[64:64 + K1, 768:],
                                in_=tw_d[64:64 + K1, 768:])
            nc.scalar.dma_start(out=w2_t, in_=w2_d)

            PASS_PAIRS = [(0, 1), (2, 3)]
            for ps in range(N_PASSES):
                prs = PASS_PAIRS[ps]
                accA = {p: apool.tile([128, 2 * CHUNK], f16,
                                      name=f"accA{p}", tag=f"aA{p % 3}")
                        for p in prs}
                accB = {p: [apool.tile([128, 2 * CHUNK], f16,
                                       name=f"accB{p}_{i}",
                                       tag=f"aB{p % 3}_{i}")
                            for i in range(2)]
                        for p in prs}
                nB = {p: 0 for p in prs}
                s2rt = {p: [] for p in prs}

                for k in range(KT):
                    kc = slice(k * 128, (k + 1) * 128)
                    for p in prs:
                        cs = p * CHUNK
                        pt = p1pool.tile([128, 2 * CHUNK], f32, tag="p1",
                                         name=f"pt{k}_{p}")
                        nc.tensor.matmul(
                            pt[:, :CHUNK], lhsT=tw_t[0:K1, kc],
                            rhs=xt_t[0:K1, cs:cs + CHUNK],
                            start=True, stop=True, tile_position=(0, 0))
                        nc.tensor.matmul(
                            pt[:, CHUNK:], lhsT=tw_t[64:64 + K1, kc],
                            rhs=xt_t[64:64 + K1, cs:cs + CHUNK],
                            start=True, stop=True, tile_position=(64, 0))
                        plan = ew_plan(p % 2)
                        if k == plan["dve_init"]:
                            nc.vector.tensor_scalar_max(
                                out=accA[p], in0=pt, scalar1=0.0)
                        elif k == plan["act_init"]:
                            nc.scalar.activation(out=accB[p][0], in_=pt,
                                                 func=relu)
                        elif k in plan["dve_stt"]:
                            nc.vector.scalar_tensor_tensor(
                                out=accA[p], in0=pt, scalar=0.0,
                                in1=accA[p], op0=mx, op1=add)
                        elif k in plan["s2_rt"]:
                            rt = rpool.tile([128, 2 * CHUNK], f16,
                                            tag=f"s2rt{k % 2}",
                                            name=f"s2rt{k}_{p}")
                            nc.scalar.activation(out=rt, in_=pt, func=relu)
                            s2rt[p].append(rt)
                        else:
                            gp = k in plan["gp_rt"]
                            # separate pools per consumer: a slow gpsimd add
                            # must not block ACT's other relu temps (WAR on
                            # a shared pool stalls the whole drain pipeline)
                            rt = rpool.tile([128, 2 * CHUNK], f16,
                                            tag="rtg" if gp else "rtv")
                            nc.scalar.activation(out=rt, in_=pt, func=relu)
                            i = nB[p]
                            eng = nc.gpsimd if gp else nc.vector
                            # gpsimd crashes on in-place; ping-pong
                            eng.tensor_tensor(
                                out=accB[p][(i + 1) % 2],
                                in0=accB[p][i % 2], in1=rt, op=add)
                            nB[p] = i + 1
                for p in prs:
                    pooled = mpool.tile([128, 2 * CHUNK], f16,
                                        name=f"pool{p}", tag="pool")
                    nc.vector.tensor_tensor(
                        out=pooled, in0=accA[p], in1=accB[p][nB[p] % 2],
                        op=add)
                    ops = p2pool.tile([D2, 2 * CHUNK], f32, tag="p2")
                    rhss = [pooled] + s2rt[p]
                    for i, rh in enumerate(rhss):
                        st = i == 0
                        sp_ = i == len(rhss) - 1
                        nc.tensor.matmul(ops[:, :CHUNK], lhsT=w2_t,
                                         rhs=rh[:, :CHUNK], start=st,
                                         stop=sp_, skip_group_check=True)
                        nc.tensor.matmul(ops[:, CHUNK:], lhsT=w2_t,
                                         rhs=rh[:, CHUNK:], start=st,
                                         stop=sp_, skip_group_check=True)
                    ot = opool.tile([D2, 2 * CHUNK], f16, name=f"ot{p}",
                                    tag="ot")
                    nc.scalar.copy(out=ot[:, :CHUNK], in_=ops[:, :CHUNK])
                    nc.vector.tensor_copy(out=ot[:, CHUNK:],
                                          in_=ops[:, CHUNK:])
                    cs = p * 2 * CHUNK
                    # split by rows across SP+ACT queues; finer on the last
                    nsplit = 4 if p == N_PAIRS - 1 else 2
                    rstep = D2 // nsplit
                    for j, r in enumerate(range(0, D2, rstep)):
                        nc.sync.dma_start(
                            out=out_d[r:r + rstep, cs:cs + 2 * CHUNK],
                            in_=ot[r:r + rstep])

    if post:
        if strip:
            _strip_redundant_ldweights(nc)
        _rebalance_matmul_waits(nc)
    return nc


def _ap_key(ap):
    return (ap.memref, ap.offset, tuple(tuple(d) for d in ap.ap))


def _strip_redundant_ldweights(nc):
    """Consecutive matmuls on the same stationary tile don't need to reload
    the PE array. Weights are tracked per row-group (row tiling keeps an
    independent stationary set per group). Any sync waits a dropped load
    held move onto the next kept PE instruction."""
    import bass_rust

    for fn in nc.m.functions:
        for blk in fn.blocks:
            insts = list(blk.instructions)
            out = []
            last_key = None
            carry_waits = []
            for inst in insts:
                tn = type(inst).__name__
                if tn == "InstLdweights":
                    tp = getattr(inst, "tile_position", None)
                    # only drop an exact repeat of the IMMEDIATELY previous
                    # load: an intervening other-row-group load invalidates
                    # this group's stationary on real HW
                    key = (tuple(tp) if tp else None, _ap_key(inst.ins[0]))
                    if key == last_key:
                        si = inst.sync_info
                        if si is not None:
                            carry_waits.extend(si.on_wait)
                            assert not si.on_update, (
                                "won't drop ldweights holding sem updates")
                        continue  # drop the instruction
                    last_key = key
                elif tn == "InstMatmult" and carry_waits:
                    si = inst.sync_info
                    waits = list(si.on_wait) if si else []
                    ups = list(si.on_update) if si else []
                    inst.sync_info = bass_rust.SyncInfo(
                        on_wait=carry_waits + waits, on_update=ups)
                    carry_waits = []
                out.append(inst)
            assert not carry_waits
            if len(out) != len(insts):
                blk.instructions = out


def _rebalance_matmul_waits(nc):
    """Walrus' TPB ISA structs accept only one sync-wait per instruction on
    the compute engines, but Tile can attach several. Keep one wait on the
    instruction and move the excess onto freshly inserted same-engine NoOps
    just before it (same in-order queue => same or stronger ordering).
    Never park waits on Ldweights: a wait there blocks the PE's
    pull-ahead that otherwise hides the weight load entirely."""
    import bass_rust

    exempt = {"InstEventSemaphore", "InstUnconditionalBranch",
              "InstCall", "InstISA", "InstNoOp"}
    nop_ctr = [0]
    for fn in nc.m.functions:
        for blk in fn.blocks:
            insts = list(blk.instructions)
            out = []
            for inst in insts:
                tn = type(inst).__name__
                si = inst.sync_info
                nw = len(si.on_wait) if si is not None else 0
                if tn in exempt or nw <= 1:
                    out.append(inst)
                    continue
                waits = list(si.on_wait)
                moved, kept = waits[:-1], waits[-1:]
                for w in moved:
                    nop_ctr[0] += 1
                    nop = mybir.InstNoOp(
                        name=f"I-waitnop-{nop_ctr[0]}", ins=[], outs=[])
                    nop.engine = inst.engine
                    nop.sync_info = bass_rust.SyncInfo(
                        on_wait=[w], on_update=[])
                    out.append(nop)
                inst.sync_info = bass_rust.SyncInfo(
                    on_wait=kept, on_update=list(si.on_update))
                out.append(inst)
            if len(out) != len(insts):
                blk.instructions = out


_NC_CACHE = None


def _get_nc():
    global _NC_CACHE
    if _NC_CACHE is None:
        import os
        strip = os.environ.get("KERNEL_NO_STRIP", "0") != "1"
        _NC_CACHE = build_bass(strip=strip)
    return _NC_CACHE


def make_in_maps(hand_landmarks, W1, b1, W2, b2, np_dt=np.float16):
    tw, w2stack = fold_weights(W1, b1, W2)
    twp = np.zeros((128, M1), np_dt)
    twp[0:K1] = tw
    twp[64:64 + K1] = tw
    w2stack = w2stack.astype(np_dt)
    x = np.asarray(hand_landmarks, np.float32).reshape(G, NNODE * CIN)
    xt = np.empty((K1, G), np.float32)
    xt[: NNODE * CIN] = x.T
    xt[K1 - 1] = 1.0
    in_maps = []
    for i in range(N_CORES):
        xc = xt[:, i * G_CORE:(i + 1) * G_CORE]
        xtp = np.zeros((128, G_CORE // 2), np_dt)
        # even chunks -> partitions 0:43, odd chunks -> 64:107
        for p in range(N_PAIRS):
            xtp[0:K1, p * CHUNK:(p + 1) * CHUNK] = \
                xc[:, (2 * p) * CHUNK:(2 * p + 1) * CHUNK]
            xtp[64:64 + K1, p * CHUNK:(p + 1) * CHUNK] = \
                xc[:, (2 * p + 1) * CHUNK:(2 * p + 2) * CHUNK]
        in_maps.append({"xt": xtp, "tw": twp, "w2": w2stack})
    return in_maps


def gather_out(results, b2):
    full = np.concatenate([results[i]["out"] for i in range(N_CORES)], axis=1)
    out = full.T.astype(np.float32) + np.asarray(b2, np.float32)[None, :]
    return np.ascontiguousarray(out).reshape(B, S, D2)


def run(in_maps, trace=False, **kw):
    res = bass_utils.run_bass_kernel_spmd(
        _get_nc(), in_maps, core_ids=list(range(N_CORES)), trace=trace, **kw)
    return res


def kernel(hand_landmarks, W1, b1, W2, b2):
    in_maps = make_in_maps(hand_landmarks, W1, b1, W2, b2)
    res = run(in_maps)
    return gather_out(res.results, b2)


# revision 44
# speedup vs baseline: 1.1822x; 1.0258x over previous
"""Trainium2 Bass kernel for nn_HandGNNEncoder (2-layer GCN on 21-node hand
graphs + mean pool), data-parallel over 8 NeuronCores.

Math restructure (exact):
  reference: h1 = relu(A @ (x @ W1) + b1); out = mean_t(A @ (h1 @ W2) + b2)
  mean-pool is linear and commutes with W2: with m[s] = column-mean of A
  (all > 0) and m folded inside the relu (m*relu(z) = relu(m*z)):
      pooled[g,f] = sum_s relu(zm[g,s,f]),   zm = m[s]*(A(xW1)+b1)[s,f]
      out[g]      = pooled[g] @ W2 + b2   (b2 added on host)
  Stage 1 (PE): zm[(s,f), g] = TW.T @ x'[g], TW[(s',c),(s,f)] =
      m[s]*A[s,s']*W1[c,f]; b1 rides a constant-1 input row.  11 k-tiles
      of 128 (s,f)-columns.  K=43 <= 64, so stage 1 runs 2x ROW-TILED:
      tile T0 (SBUF partitions 0-42) computes even chunks while tile T8
      (partitions 64-106) concurrently computes odd chunks — measured 2x
      matmul throughput.  Each (k, chunk-pair) writes one [128,1024] PSUM
      pair tile (even chunk left half / odd chunk right half, 2 banks).
  Pooling: relu+accumulate on [128,1024] pair ops: DVE scalar_tensor_tensor
      chains (accA), ACT relu -> temps added by gpsimd/DVE (accB),
      merged to f16 pooled on DVE.
  Stage 2 (PE, full 128x128): one matmul per chunk, W2STACK[p,d] =
      W2[p%64,d]; psum pair-copied to f16 by ACT; DMA out per pair.

Input packing (host): xtp [128, G_CORE/2] holds even chunks' x'T on
partitions 0-42 and odd chunks' on partitions 64-106, chunk-pair p at
columns [512p:512(p+1)).  twp duplicates TW on both partition groups.
"""

import numpy as np

import concourse.bass as bass
import concourse.mybir as mybir
import concourse.tile as tile
from concourse import bass_utils

# ---- hardcoded problem constants ----
B, S, NNODE, CIN = 64, 512, 21, 2
D1, D2 = 64, 128
G = B * S                      # 32768 graphs
N_CORES = 8
G_CORE = G // N_CORES          # 4096 graphs per core
CHUNK = 512                    # graphs per chunk (one PSUM bank)
N_CHUNKS = G_CORE // CHUNK     # 8
N_PAIRS = N_CHUNKS // 2        # 4 chunk-pairs
K1 = NNODE * CIN + 1           # 43 contraction rows (42 feats + ones row)
KT = 11                        # k-tiles: 1408 / 128
M1 = KT * 128                  # 1344 (s,f) cols + 64 zero pad
N_PASSES = 2                   # pairs per pass = 2

EDGES = np.array(
    [[0, 1], [1, 2], [2, 3], [3, 4], [0, 5], [5, 6], [6, 7], [7, 8],
     [0, 9], [9, 10], [10, 11], [11, 12], [0, 13], [13, 14], [14, 15],
     [15, 16], [0, 17], [17, 18], [18, 19], [19, 20], [5, 9], [9, 13],
     [13, 17]], dtype=np.int64)


def fold_weights(W1, b1, W2):
    """Fold adjacency + mean-pool + b1 into TW; stack W2 for stage 2."""
    W1 = np.asarray(W1, np.float32)
    b1 = np.asarray(b1, np.float32)
    W2 = np.asarray(W2, np.float32)
    A = np.eye(NNODE, dtype=np.float32)
    A[EDGES[:, 1], EDGES[:, 0]] = 1.0
    deg = A.sum(axis=1)
    dis = 1.0 / np.sqrt(deg)
    a_norm = dis[:, None] * A * dis[None, :]          # [t, s]
    m = a_norm.mean(axis=0)                           # [21], all > 0

    tw = np.zeros((K1, M1), np.float32)
    blk = np.einsum("s,st,cf->tcsf", m, a_norm, W1)   # [s'=t, c, s, f]
    tw[: NNODE * CIN, : NNODE * D1] = blk.reshape(NNODE * CIN, NNODE * D1)
    tw[K1 - 1, : NNODE * D1] = (m[:, None] * b1[None, :]).reshape(-1)

    w2stack = np.concatenate([W2, W2], axis=0)        # [128, 128]
    return tw, w2stack


def build_bass(post=True, strip=True):
    f16 = mybir.dt.float16
    f32 = mybir.dt.float32
    nc = bass.Bass("TRN2", target_bir_lowering=False, debug=False)
    # xtp packs even chunks on partitions 0:43, odd chunks on 64:107
    xt_d = nc.dram_tensor("xt", [128, G_CORE // 2], f16,
                          kind="ExternalInput").ap()
    tw_d = nc.dram_tensor("tw", [128, M1], f16, kind="ExternalInput").ap()
    w2_d = nc.dram_tensor("w2", [D2, D2], f16, kind="ExternalInput").ap()
    out_d = nc.dram_tensor("out", [D2, G_CORE], f16, kind="ExternalOutput").ap()

    relu = mybir.ActivationFunctionType.Relu
    mx = mybir.AluOpType.max
    add = mybir.AluOpType.add
    PPP = N_PAIRS // N_PASSES             # pairs per pass

    # EW assignment (chunk-pair granularity, [128,1024] ops), phase-flipped
    # by pair parity so each k-step feeds ONE pair-op to DVE and one to ACT
    # (same-engine k-sets leave engines alternately idle/overloaded):
    #   pair phase 0: DVE drains k even (k0 init-max, rest stt into accA);
    #                 ACT drains k odd (k1 init-relu accB, k3/k5 -> gpsimd
    #                 ping-pong adds, k7/k9 -> stage-2 direct matmuls)
    #   pair phase 1: mirrored (DVE odd + k10, ACT even, k2/k4 gp adds,
    #                 k6/k8 stage-2 direct)
    #   merges (pooled = accA+accB) on gpsimd, copies on ACT
    def ew_plan(ph):
        return dict(dve_init=0, dve_stt=(2, 4, 6, 8),
                    act_init=1, gp_rt=(3, 5), dve_rt=(7,), s2_rt=(9, 10))

    with tile.TileContext(nc) as tc:
        with (
            tc.tile_pool(name="w", bufs=1) as wpool,
            tc.tile_pool(name="a", bufs=1) as apool,
            tc.tile_pool(name="r", bufs=8) as rpool,
            tc.tile_pool(name="m", bufs=4) as mpool,
            tc.tile_pool(name="o", bufs=4) as opool,
            tc.tile_pool(name="p1", bufs=3, space="PSUM") as p1pool,
            tc.tile_pool(name="p2", bufs=2, space="PSUM") as p2pool,
        ):
            tw_t = wpool.tile([128, M1], f16, tag="tw")
            xt_t = wpool.tile([128, G_CORE // 2], f16, tag="xt")
            w2_t = wpool.tile([D2, D2], f16, tag="w2")
            # one dma_start = one DMA engine (~22.5 GB/s); parallelize by
            # issuing many.  SP: per-pair xt slices (top+bottom halves);
            # ACT: tw + w2.
            for p in range(N_PAIRS):
                cs = p * CHUNK
                nc.sync.dma_start(out=xt_t[0:K1, cs:cs + CHUNK],
                                  in_=xt_d[0:K1, cs:cs + CHUNK])
                nc.sync.dma_start(out=xt_t[64:64 + K1, cs:cs + CHUNK],
                                  in_=xt_d[64:64 + K1, cs:cs + CHUNK])
            nc.scalar.dma_start(out=tw_t[0:K1, :128], in_=tw_d[0:K1, :128])
            nc.scalar.dma_start(out=tw_t[64:64 + K1, :128],
                                in_=tw_d[64:64 + K1, :128])
            nc.scalar.dma_start(out=tw_t[0:K1, 128:768],
                                in_=tw_d[0:K1, 128:768])
            nc.scalar.dma_start(out=tw_t[64:64 + K1, 128:768],
                                in_=tw_d[64:64 + K1, 128:768])
            nc.scalar.dma_start(out=tw_t[0:K1, 768:], in_=tw_d[0:K1, 768:])
            nc.scalar.dma_start(out=tw_t[64:64 + K1, 768:],
                                in_=tw_d[64:64 + K1, 768:])
            nc.scalar.dma_start(out=w2_t, in_=w2_d)

            PASS_PAIRS = [(0, 1), (2, 3)]
            for ps in range(N_PASSES):
                prs = PASS_PAIRS[ps]
                accA = {p: apool.tile([128, 2 * CHUNK], f16,
                                      name=f"accA{p}", tag=f"aA{p % 3}")
                        for p in prs}
                accB = {p: [apool.tile([128, 2 * CHUNK], f16,
                                       name=f"accB{p}_{i}",
                                       tag=f"aB{p % 3}_{i}")
                            for i in range(2)]
                        for p in prs}
                nB = {p: 0 for p in prs}
                s2rt = {p: [] for p in prs}

                for k in range(KT):
                    kc = slice(k * 128, (k + 1) * 128)
                    for p in prs:
                        cs = p * CHUNK
                        pt = p1pool.tile([128, 2 * CHUNK], f32, tag="p1",
                                         name=f"pt{k}_{p}")
                        nc.tensor.matmul(
                            pt[:, :CHUNK], lhsT=tw_t[0:K1, kc],
                            rhs=xt_t[0:K1, cs:cs + CHUNK],
                            start=True, stop=True, tile_position=(0, 0))
                        nc.tensor.matmul(
                            pt[:, CHUNK:], lhsT=tw_t[64:64 + K1, kc],
                            rhs=xt_t[64:64 + K1, cs:cs + CHUNK],
                            start=True, stop=True, tile_position=(64, 0))
                        plan = ew_plan(p % 2)
                        if k == plan["dve_init"]:
                            nc.vector.tensor_scalar_max(
                                out=accA[p], in0=pt, scalar1=0.0)
                        elif k == plan["act_init"]:
                            nc.scalar.activation(out=accB[p][0], in_=pt,
                                                 func=relu)
                        elif k in plan["dve_stt"]:
                            nc.vector.scalar_tensor_tensor(
                                out=accA[p], in0=pt, scalar=0.0,
                                in1=accA[p], op0=mx, op1=add)
                        elif k in plan["s2_rt"]:
                            rt = rpool.tile([128, 2 * CHUNK], f16,
                                            tag=f"s2rt{k % 2}",
                                            name=f"s2rt{k}_{p}")
                            nc.scalar.activation(out=rt, in_=pt, func=relu)
                            s2rt[p].append(rt)
                        else:
                            gp = k in plan["gp_rt"]
                            # separate pools per consumer: a slow gpsimd add
                            # must not block ACT's other relu temps (WAR on
                            # a shared pool stalls the whole drain pipeline)
                            rt = rpool.tile([128, 2 * CHUNK], f16,
                                            tag="rtg" if gp else "rtv")
                            nc.scalar.activation(out=rt, in_=pt, func=relu)
                            i = nB[p]
                            eng = nc.gpsimd if gp else nc.vector
                            # gpsimd crashes on in-place; ping-pong
                            eng.tensor_tensor(
                                out=accB[p][(i + 1) % 2],
                                in0=accB[p][i % 2], in1=rt, op=add)
                            nB[p] = i + 1
                for p in prs:
                    pooled = mpool.tile([128, 2 * CHUNK], f16,
                                        name=f"pool{p}", tag="pool")
                    nc.vector.tensor_tensor(
                        out=pooled, in0=accA[p], in1=accB[p][nB[p] % 2],
                        op=add)
                    opsL = p2pool.tile([D2, CHUNK], f32, tag="p2",
                                       name=f"opsL{p}")
                    opsR = p2pool.tile([D2, CHUNK], f32, tag="p2",
                                       name=f"opsR{p}")
                    rhss = [pooled] + s2rt[p]
                    for i, rh in enumerate(rhss):
                        st = i == 0
                        sp_ = i == len(rhss) - 1
                        nc.tensor.matmul(opsL, lhsT=w2_t,
                                         rhs=rh[:, :CHUNK], start=st,
                                         stop=sp_, skip_group_check=True)
                        nc.tensor.matmul(opsR, lhsT=w2_t,
                                         rhs=rh[:, CHUNK:], start=st,
                                         stop=sp_, skip_group_check=True)
                    ot = opool.tile([D2, 2 * CHUNK], f16, name=f"ot{p}",
                                    tag="ot")
                    nc.scalar.copy(out=ot[:, :CHUNK], in_=opsL)
                    nc.vector.tensor_copy(out=ot[:, CHUNK:], in_=opsR)
                    cs = p * 2 * CHUNK
                    # split by rows across SP+ACT queues; finer on the last
                    nsplit = 4 if p == N_PAIRS - 1 else 2
                    rstep = D2 // nsplit
                    for j, r in enumerate(range(0, D2, rstep)):
                        eng = nc.sync if j % 2 == 0 else nc.gpsimd
                        eng.dma_start(
                            out=out_d[r:r + rstep, cs:cs + 2 * CHUNK],
                            in_=ot[r:r + rstep])

    if post:
        if strip:
            _strip_redundant_ldweights(nc)
        _rebalance_matmul_waits(nc)
    return nc


def _ap_key(ap):
    return (ap.memref, ap.offset, tuple(tuple(d) for d in ap.ap))


def _strip_redundant_ldweights(nc):
    """Consecutive matmuls on the same stationary tile don't need to reload
    the PE array. Weights are tracked per row-group (row tiling keeps an
    independent stationary set per group). Any sync waits a dropped load
    held move onto the next kept PE instruction."""
    import bass_rust

    for fn in nc.m.functions:
        for blk in fn.blocks:
            insts = list(blk.instructions)
            out = []
            last_key = None
            carry_waits = []
            for inst in insts:
                tn = type(inst).__name__
                if tn == "InstLdweights":
                    tp = getattr(inst, "tile_position", None)
                    # only drop an exact repeat of the IMMEDIATELY previous
                    # load: an intervening other-row-group load invalidates
                    # this group's stationary on real HW
                    key = (tuple(tp) if tp else None, _ap_key(inst.ins[0]))
                    if key == last_key:
                        si = inst.sync_info
                        if si is not None:
                            carry_waits.extend(si.on_wait)
                            assert not si.on_update, (
                                "won't drop ldweights holding sem updates")
                        continue  # drop the instruction
                    last_key = key
                elif tn == "InstMatmult" and carry_waits:
                    si = inst.sync_info
                    waits = list(si.on_wait) if si else []
                    ups = list(si.on_update) if si else []
                    inst.sync_info = bass_rust.SyncInfo(
                        on_wait=carry_waits + waits, on_update=ups)
                    carry_waits = []
                out.append(inst)
            assert not carry_waits
            if len(out) != len(insts):
                blk.instructions = out


def _rebalance_matmul_waits(nc):
    """Walrus' TPB ISA structs accept only one sync-wait per instruction on
    the compute engines, but Tile can attach several. Keep one wait on the
    instruction and move the excess onto freshly inserted same-engine NoOps
    just before it (same in-order queue => same or stronger ordering).
    Never park waits on Ldweights: a wait there blocks the PE's
    pull-ahead that otherwise hides the weight load entirely."""
    import bass_rust

    exempt = {"InstEventSemaphore", "InstUnconditionalBranch",
              "InstCall", "InstISA", "InstNoOp"}
    nop_ctr = [0]
    for fn in nc.m.functions:
        for blk in fn.blocks:
            insts = list(blk.instructions)
            out = []
            for inst in insts:
                tn = type(inst).__name__
                si = inst.sync_info
                nw = len(si.on_wait) if si is not None else 0
                if tn in exempt or nw <= 1:
                    out.append(inst)
                    continue
                waits = list(si.on_wait)
                moved, kept = waits[:-1], waits[-1:]
                for w in moved:
                    nop_ctr[0] += 1
                    nop = mybir.InstNoOp(
                        name=f"I-waitnop-{nop_ctr[0]}", ins=[], outs=[])
                    nop.engine = inst.engine
                    nop.sync_info = bass_rust.SyncInfo(
                        on_wait=[w], on_update=[])
                    out.append(nop)
                inst.sync_info = bass_rust.SyncInfo(
                    on_wait=kept, on_update=list(si.on_update))
                out.append(inst)
            if len(out) != len(insts):
                blk.instructions = out


_NC_CACHE = None


def _get_nc():
    global _NC_CACHE
    if _NC_CACHE is None:
        import os
        strip = os.environ.get("KERNEL_NO_STRIP", "0") != "1"
        _NC_CACHE = build_bass(strip=strip)
    return _NC_CACHE


def make_in_maps(hand_landmarks, W1, b1, W2, b2, np_dt=np.float16):
    tw, w2stack = fold_weights(W1, b1, W2)
    twp = np.zeros((128, M1), np_dt)
    twp[0:K1] = tw
    twp[64:64 + K1] = tw
    w2stack = w2stack.astype(np_dt)
    x = np.asarray(hand_landmarks, np.float32).reshape(G, NNODE * CIN)
    xt = np.empty((K1, G), np.float32)
    xt[: NNODE * CIN] = x.T
    xt[K1 - 1] = 1.0
    in_maps = []
    for i in range(N_CORES):
        xc = xt[:, i * G_CORE:(i + 1) * G_CORE]
        xtp = np.zeros((128, G_CORE // 2), np_dt)
        # even chunks -> partitions 0:43, odd chunks -> 64:107
        for p in range(N_PAIRS):
            xtp[0:K1, p * CHUNK:(p + 1) * CHUNK] = \
                xc[:, (2 * p) * CHUNK:(2 * p + 1) * CHUNK]
            xtp[64:64 + K1, p * CHUNK:(p + 1) * CHUNK] = \
                xc[:, (2 * p + 1) * CHUNK:(2 * p + 2) * CHUNK]
        in_maps.append({"xt": xtp, "tw": twp, "w2": w2stack})
    return in_maps


def gather_out(results, b2):
    full = np.concatenate([results[i]["out"] for i in range(N_CORES)], axis=1)
    out = full.T.astype(np.float32) + np.asarray(b2, np.float32)[None, :]
    return np.ascontiguousarray(out).reshape(B, S, D2)


def run(in_maps, trace=False, **kw):
    res = bass_utils.run_bass_kernel_spmd(
        _get_nc(), in_maps, core_ids=list(range(N_CORES)), trace=trace, **kw)
    return res


def kernel(hand_landmarks, W1, b1, W2, b2):
    in_maps = make_in_maps(hand_landmarks, W1, b1, W2, b2)
    res = run(in_maps)
    return gather_out(res.results, b2)


# revision 48
# speedup vs baseline: 1.2210x; 1.0329x over previous
"""Trainium2 Bass kernel for nn_HandGNNEncoder (2-layer GCN on 21-node hand
graphs + mean pool), data-parallel over 8 NeuronCores.

Math restructure (exact):
  reference: h1 = relu(A @ (x @ W1) + b1); out = mean_t(A @ (h1 @ W2) + b2)
  mean-pool is linear and commutes with W2: with m[s] = column-mean of A
  (all > 0) and m folded inside the relu (m*relu(z) = relu(m*z)):
      pooled[g,f] = sum_s relu(zm[g,s,f]),   zm = m[s]*(A(xW1)+b1)[s,f]
      out[g]      = pooled[g] @ W2 + b2   (b2 added on host)
  Stage 1 (PE): zm[(s,f), g] = TW.T @ x'[g], TW[(s',c),(s,f)] =
      m[s]*A[s,s']*W1[c,f]; b1 rides a constant-1 input row.  11 k-tiles
      of 128 (s,f)-columns.  K=43 <= 64, so stage 1 runs 2x ROW-TILED:
      tile T0 (SBUF partitions 0-42) computes even chunks while tile T8
      (partitions 64-106) concurrently computes odd chunks — measured 2x
      matmul throughput.  Each (k, chunk-pair) writes one [128,1024] PSUM
      pair tile (even chunk left half / odd chunk right half, 2 banks).
  Pooling: relu+accumulate on [128,1024] pair ops: DVE scalar_tensor_tensor
      chains (accA), ACT relu -> temps added by gpsimd/DVE (accB),
      merged to f16 pooled on DVE.
  Stage 2 (PE, full 128x128): one matmul per chunk, W2STACK[p,d] =
      W2[p%64,d]; psum pair-copied to f16 by ACT; DMA out per pair.

Input packing (host): xtp [128, G_CORE/2] holds even chunks' x'T on
partitions 0-42 and odd chunks' on partitions 64-106, chunk-pair p at
columns [512p:512(p+1)).  twp duplicates TW on both partition groups.
"""

import numpy as np

import concourse.bass as bass
import concourse.mybir as mybir
import concourse.tile as tile
from concourse import bass_utils

# ---- hardcoded problem constants ----
B, S, NNODE, CIN = 64, 512, 21, 2
D1, D2 = 64, 128
G = B * S                      # 32768 graphs
N_CORES = 8
G_CORE = G // N_CORES          # 4096 graphs per core
CHUNK = 512                    # graphs per chunk (one PSUM bank)
N_CHUNKS = G_CORE // CHUNK     # 8
N_PAIRS = N_CHUNKS // 2        # 4 chunk-pairs
K1 = NNODE * CIN + 1           # 43 contraction rows (42 feats + ones row)
KT = 11                        # k-tiles: 1408 / 128
M1 = KT * 128                  # 1344 (s,f) cols + 64 zero pad
N_PASSES = 2                   # pairs per pass = 2

EDGES = np.array(
    [[0, 1], [1, 2], [2, 3], [3, 4], [0, 5], [5, 6], [6, 7], [7, 8],
     [0, 9], [9, 10], [10, 11], [11, 12], [0, 13], [13, 14], [14, 15],
     [15, 16], [0, 17], [17, 18], [18, 19], [19, 20], [5, 9], [9, 13],
     [13, 17]], dtype=np.int64)


def fold_weights(W1, b1, W2):
    """Fold adjacency + mean-pool + b1 into TW; stack W2 for stage 2."""
    W1 = np.asarray(W1, np.float32)
    b1 = np.asarray(b1, np.float32)
    W2 = np.asarray(W2, np.float32)
    A = np.eye(NNODE, dtype=np.float32)
    A[EDGES[:, 1], EDGES[:, 0]] = 1.0
    deg = A.sum(axis=1)
    dis = 1.0 / np.sqrt(deg)
    a_norm = dis[:, None] * A * dis[None, :]          # [t, s]
    m = a_norm.mean(axis=0)                           # [21], all > 0

    tw = np.zeros((K1, M1), np.float32)
    blk = np.einsum("s,st,cf->tcsf", m, a_norm, W1)   # [s'=t, c, s, f]
    tw[: NNODE * CIN, : NNODE * D1] = blk.reshape(NNODE * CIN, NNODE * D1)
    tw[K1 - 1, : NNODE * D1] = (m[:, None] * b1[None, :]).reshape(-1)

    w2stack = np.concatenate([W2, W2], axis=0)        # [128, 128]
    return tw, w2stack


def build_bass(post=True, strip=True):
    f16 = mybir.dt.float16
    f32 = mybir.dt.float32
    nc = bass.Bass("TRN2", target_bir_lowering=False, debug=False)
    # xtp packs even chunks on partitions 0:43, odd chunks on 64:107
    xt_d = nc.dram_tensor("xt", [128, G_CORE // 2], f16,
                          kind="ExternalInput").ap()
    tw_d = nc.dram_tensor("tw", [128, M1], f16, kind="ExternalInput").ap()
    w2_d = nc.dram_tensor("w2", [D2, D2], f16, kind="ExternalInput").ap()
    out_d = nc.dram_tensor("out", [D2, G_CORE], f16, kind="ExternalOutput").ap()

    relu = mybir.ActivationFunctionType.Relu
    mx = mybir.AluOpType.max
    add = mybir.AluOpType.add
    PPP = N_PAIRS // N_PASSES             # pairs per pass

    # EW assignment (chunk-pair granularity, [128,1024] ops), phase-flipped
    # by pair parity so each k-step feeds ONE pair-op to DVE and one to ACT
    # (same-engine k-sets leave engines alternately idle/overloaded):
    #   pair phase 0: DVE drains k even (k0 init-max, rest stt into accA);
    #                 ACT drains k odd (k1 init-relu accB, k3/k5 -> gpsimd
    #                 ping-pong adds, k7/k9 -> stage-2 direct matmuls)
    #   pair phase 1: mirrored (DVE odd + k10, ACT even, k2/k4 gp adds,
    #                 k6/k8 stage-2 direct)
    #   merges (pooled = accA+accB) on gpsimd, copies on ACT
    def ew_plan(ph):
        return dict(dve_init=0, dve_stt=(2, 4, 6, 8),
                    act_init=1, gp_rt=(3,), dve_rt=(5, 7), s2_rt=(9, 10))

    with tile.TileContext(nc) as tc:
        with (
            tc.tile_pool(name="w", bufs=1) as wpool,
            tc.tile_pool(name="a", bufs=1) as apool,
            tc.tile_pool(name="r", bufs=8) as rpool,
            tc.tile_pool(name="m", bufs=4) as mpool,
            tc.tile_pool(name="o", bufs=4) as opool,
            tc.tile_pool(name="p1", bufs=3, space="PSUM") as p1pool,
            tc.tile_pool(name="p2", bufs=2, space="PSUM") as p2pool,
        ):
            tw_t = wpool.tile([128, M1], f16, tag="tw")
            xt_t = wpool.tile([128, G_CORE // 2], f16, tag="xt")
            w2_t = wpool.tile([D2, D2], f16, tag="w2")
            # one dma_start = one DMA engine (~22.5 GB/s); parallelize by
            # issuing many.  SP: per-pair xt slices (top+bottom halves);
            # ACT: tw + w2.
            for p in range(N_PAIRS):
                cs = p * CHUNK
                nc.sync.dma_start(out=xt_t[0:K1, cs:cs + CHUNK],
                                  in_=xt_d[0:K1, cs:cs + CHUNK])
                nc.sync.dma_start(out=xt_t[64:64 + K1, cs:cs + CHUNK],
                                  in_=xt_d[64:64 + K1, cs:cs + CHUNK])
            nc.scalar.dma_start(out=tw_t[0:K1, :128], in_=tw_d[0:K1, :128])
            nc.scalar.dma_start(out=tw_t[64:64 + K1, :128],
                                in_=tw_d[64:64 + K1, :128])
            nc.scalar.dma_start(out=tw_t[0:K1, 128:768],
                                in_=tw_d[0:K1, 128:768])
            nc.scalar.dma_start(out=tw_t[64:64 + K1, 128:768],
                                in_=tw_d[64:64 + K1, 128:768])
            nc.scalar.dma_start(out=tw_t[0:K1, 768:], in_=tw_d[0:K1, 768:])
            nc.scalar.dma_start(out=tw_t[64:64 + K1, 768:],
                                in_=tw_d[64:64 + K1, 768:])
            nc.scalar.dma_start(out=w2_t, in_=w2_d)

            PASS_PAIRS = [(0, 1), (2, 3)]
            for ps in range(N_PASSES):
                prs = PASS_PAIRS[ps]
                accA = {p: apool.tile([128, 2 * CHUNK], f16,
                                      name=f"accA{p}", tag=f"aA{p % 3}")
                        for p in prs}
                accB = {p: [apool.tile([128, 2 * CHUNK], f16,
                                       name=f"accB{p}_{i}",
                                       tag=f"aB{p % 3}_{i}")
                            for i in range(2)]
                        for p in prs}
                nB = {p: 0 for p in prs}
                s2rt = {p: [] for p in prs}

                for k in range(KT):
                    kc = slice(k * 128, (k + 1) * 128)
                    for p in prs:
                        cs = p * CHUNK
                        pt = p1pool.tile([128, 2 * CHUNK], f32, tag="p1",
                                         name=f"pt{k}_{p}")
                        nc.tensor.matmul(
                            pt[:, :CHUNK], lhsT=tw_t[0:K1, kc],
                            rhs=xt_t[0:K1, cs:cs + CHUNK],
                            start=True, stop=True, tile_position=(0, 0))
                        nc.tensor.matmul(
                            pt[:, CHUNK:], lhsT=tw_t[64:64 + K1, kc],
                            rhs=xt_t[64:64 + K1, cs:cs + CHUNK],
                            start=True, stop=True, tile_position=(64, 0))
                        plan = ew_plan(p % 2)
                        if k == plan["dve_init"]:
                            nc.vector.tensor_scalar_max(
                                out=accA[p], in0=pt, scalar1=0.0)
                        elif k == plan["act_init"]:
                            nc.scalar.activation(out=accB[p][0], in_=pt,
                                                 func=relu)
                        elif k in plan["dve_stt"]:
                            nc.vector.scalar_tensor_tensor(
                                out=accA[p], in0=pt, scalar=0.0,
                                in1=accA[p], op0=mx, op1=add)
                        elif k in plan["s2_rt"]:
                            rt = rpool.tile([128, 2 * CHUNK], f16,
                                            tag=f"s2rt{k % 2}",
                                            name=f"s2rt{k}_{p}")
                            nc.scalar.activation(out=rt, in_=pt, func=relu)
                            s2rt[p].append(rt)
                        else:
                            gp = k in plan["gp_rt"]
                            # separate pools per consumer: a slow gpsimd add
                            # must not block ACT's other relu temps (WAR on
                            # a shared pool stalls the whole drain pipeline)
                            rt = rpool.tile([128, 2 * CHUNK], f16,
                                            tag="rtg" if gp else "rtv")
                            nc.scalar.activation(out=rt, in_=pt, func=relu)
                            i = nB[p]
                            eng = nc.gpsimd if gp else nc.vector
                            # gpsimd crashes on in-place; ping-pong
                            eng.tensor_tensor(
                                out=accB[p][(i + 1) % 2],
                                in0=accB[p][i % 2], in1=rt, op=add)
                            nB[p] = i + 1
                for p in prs:
                    pooled = mpool.tile([128, 2 * CHUNK], f16,
                                        name=f"pool{p}", tag="pool")
                    nc.vector.tensor_tensor(
                        out=pooled, in0=accA[p], in1=accB[p][nB[p] % 2],
                        op=add)
                    opsL = p2pool.tile([D2, CHUNK], f32, tag="p2",
                                       name=f"opsL{p}")
                    opsR = p2pool.tile([D2, CHUNK], f32, tag="p2",
                                       name=f"opsR{p}")
                    rhss = [pooled] + s2rt[p]
                    for i, rh in enumerate(rhss):
                        st = i == 0
                        sp_ = i == len(rhss) - 1
                        nc.tensor.matmul(opsL, lhsT=w2_t,
                                         rhs=rh[:, :CHUNK], start=st,
                                         stop=sp_, skip_group_check=True)
                        nc.tensor.matmul(opsR, lhsT=w2_t,
                                         rhs=rh[:, CHUNK:], start=st,
                                         stop=sp_, skip_group_check=True)
                    ot = opool.tile([D2, 2 * CHUNK], f16, name=f"ot{p}",
                                    tag="ot")
                    nc.scalar.copy(out=ot[:, :CHUNK], in_=opsL)
                    nc.vector.tensor_copy(out=ot[:, CHUNK:], in_=opsR)
                    cs = p * 2 * CHUNK
                    # split by rows across SP+ACT queues; finer on the last
                    nsplit = 4 if p == N_PAIRS - 1 else 2
                    rstep = D2 // nsplit
                    for j, r in enumerate(range(0, D2, rstep)):
                        eng = nc.sync if j % 2 == 0 else nc.gpsimd
                        eng.dma_start(
                            out=out_d[r:r + rstep, cs:cs + 2 * CHUNK],
                            in_=ot[r:r + rstep])

    if post:
        if strip:
            _strip_redundant_ldweights(nc)
        _rebalance_matmul_waits(nc)
    return nc


def _ap_key(ap):
    return (ap.memref, ap.offset, tuple(tuple(d) for d in ap.ap))


def _strip_redundant_ldweights(nc):
    """Consecutive matmuls on the same stationary tile don't need to reload
    the PE array. Weights are tracked per row-group (row tiling keeps an
    independent stationary set per group). Any sync waits a dropped load
    held move onto the next kept PE instruction."""
    import bass_rust

    for fn in nc.m.functions:
        for blk in fn.blocks:
            insts = list(blk.instructions)
            out = []
            last_key = None
            carry_waits = []
            for inst in insts:
                tn = type(inst).__name__
                if tn == "InstLdweights":
                    tp = getattr(inst, "tile_position", None)
                    # only drop an exact repeat of the IMMEDIATELY previous
                    # load: an intervening other-row-group load invalidates
                    # this group's stationary on real HW
                    key = (tuple(tp) if tp else None, _ap_key(inst.ins[0]))
                    if key == last_key:
                        si = inst.sync_info
                        if si is not None:
                            carry_waits.extend(si.on_wait)
                            assert not si.on_update, (
                                "won't drop ldweights holding sem updates")
                        continue  # drop the instruction
                    last_key = key
                elif tn == "InstMatmult" and carry_waits:
                    si = inst.sync_info
                    waits = list(si.on_wait) if si else []
                    ups = list(si.on_update) if si else []
                    inst.sync_info = bass_rust.SyncInfo(
                        on_wait=carry_waits + waits, on_update=ups)
                    carry_waits = []
                out.append(inst)
            assert not carry_waits
            if len(out) != len(insts):
                blk.instructions = out


def _rebalance_matmul_waits(nc):
    """Walrus' TPB ISA structs accept only one sync-wait per instruction on
    the compute engines, but Tile can attach several. Keep one wait on the
    instruction and move the excess onto freshly inserted same-engine NoOps
    just before it (same in-order queue => same or stronger ordering).
    Never park waits on Ldweights: a wait there blocks the PE's
    pull-ahead that otherwise hides the weight load entirely."""
    import bass_rust

    exempt = {"InstEventSemaphore", "InstUnconditionalBranch",
              "InstCall", "InstISA", "InstNoOp"}
    nop_ctr = [0]
    for fn in nc.m.functions:
        for blk in fn.blocks:
            insts = list(blk.instructions)
            out = []
            for inst in insts:
                tn = type(inst).__name__
                si = inst.sync_info
                nw = len(si.on_wait) if si is not None else 0
                if tn in exempt or nw <= 1:
                    out.append(inst)
                    continue
                waits = list(si.on_wait)
                moved, kept = waits[:-1], waits[-1:]
                for w in moved:
                    nop_ctr[0] += 1
                    nop = mybir.InstNoOp(
                        name=f"I-waitnop-{nop_ctr[0]}", ins=[], outs=[])
                    nop.engine = inst.engine
                    nop.sync_info = bass_rust.SyncInfo(
                        on_wait=[w], on_update=[])
                    out.append(nop)
                inst.sync_info = bass_rust.SyncInfo(
                    on_wait=kept, on_update=list(si.on_update))
                out.append(inst)
            if len(out) != len(insts):
                blk.instructions = out


_NC_CACHE = None


def _get_nc():
    global _NC_CACHE
    if _NC_CACHE is None:
        import os
        strip = os.environ.get("KERNEL_NO_STRIP", "0") != "1"
        _NC_CACHE = build_bass(strip=strip)
    return _NC_CACHE


def make_in_maps(hand_landmarks, W1, b1, W2, b2, np_dt=np.float16):
    tw, w2stack = fold_weights(W1, b1, W2)
    twp = np.zeros((128, M1), np_dt)
    twp[0:K1] = tw
    twp[64:64 + K1] = tw
    w2stack = w2stack.astype(np_dt)
    x = np.asarray(hand_landmarks, np.float32).reshape(G, NNODE * CIN)
    xt = np.empty((K1, G), np.float32)
    xt[: NNODE * CIN] = x.T
    xt[K1 - 1] = 1.0
    in_maps = []
    for i in range(N_CORES):
        xc = xt[:, i * G_CORE:(i + 1) * G_CORE]
        xtp = np.zeros((128, G_CORE // 2), np_dt)
        # even chunks -> partitions 0:43, odd chunks -> 64:107
        for p in range(N_PAIRS):
            xtp[0:K1, p * CHUNK:(p + 1) * CHUNK] = \
                xc[:, (2 * p) * CHUNK:(2 * p + 1) * CHUNK]
            xtp[64:64 + K1, p * CHUNK:(p + 1) * CHUNK] = \
                xc[:, (2 * p + 1) * CHUNK:(2 * p + 2) * CHUNK]
        in_maps.append({"xt": xtp, "tw": twp, "w2": w2stack})
    return in_maps


def gather_out(results, b2):
    full = np.concatenate([results[i]["out"] for i in range(N_CORES)], axis=1)
    out = full.T.astype(np.float32) + np.asarray(b2, np.float32)[None, :]
    return np.ascontiguousarray(out).reshape(B, S, D2)


def run(in_maps, trace=False, **kw):
    res = bass_utils.run_bass_kernel_spmd(
        _get_nc(), in_maps, core_ids=list(range(N_CORES)), trace=trace, **kw)
    return res


def kernel(hand_landmarks, W1, b1, W2, b2):
    in_maps = make_in_maps(hand_landmarks, W1, b1, W2, b2)
    res = run(in_maps)
    return gather_out(res.results, b2)
